# revision 1
# baseline (speedup 1.0000x reference)
"""Trainium2 Bass kernel for nn_CalibrationNetwork (MoE-routed 3-layer MLP + softmax).

Strategy
--------
Host side (numpy): sort samples by judge id, pad each judge group to a
multiple of 256 samples ("supertile"), distribute supertiles round-robin
over 8 cores (20 slots/core covers the worst case sum_j ceil(c_j/256) <= 160).
All judge-specific tables are combined with the shared ones on the host
(W1+W1_a[j] etc.), so the device never gathers. Questions are processed as
block-diagonal pairs (two 64-wide blocks fill the 128 partitions/PE columns);
the odd 7th question (q6) of the two supertiles in a macro-iteration shares
one block (s=0 half = first tile's q6, s=1 half = second tile's q6, with the
two tiles' experts mixed in the block-diagonal weights).

Per supertile the host emits, merged into two HBM streams:
  - xw [12, XW] bf16: cols 0..1024 = transposed x with bias-ones rows (rows
    s*6+d; d=0 is ones), pair p in cols p*256; cols 1024..1616 = L1
    stationary blocks (rows s*6+d, cols p*128+s*64+h) with partition-0 cols
    1536..1696 holding both tiles' L3 bias rows b3 (o-major, (blk,p,s,o)).
  - wb [128, 300] bf16: cols 0..128 L2 block-diag stationary [[W2,0],[0,W2]];
    cols 128..168 L3 moving operand (V block per pair, cols p*10+s*5+o);
    cols 168..172 the L2 bias columns as raw f32 bits (bitcast on device,
    own bias then q6-mixed bias); cols 172..300 the q6-mixed L2 stationary
    [[W2(j0),0],[0,W2(j1)]].

Device (per macro = two supertiles; all engines pipelined by Tile):
  bias: 1 matmul ones_col^T @ b3rows -> psum3 [128, 160]        (K=1, N=160)
  L1: 7 matmuls  psum1[(s,h1), (p,b)] += wa_p^T @ xt_p          (K=12, M=128, N=256)
  relu1 (DVE):   z1 = max(psum1, 0) -> bf16                     (bias via ones row)
  L2: 7 matmuls  psum2[(s,h2), (p,b)] += w2blk^T @ z1_p         (K=128, M=128, N=256)
  relu2 (ACT):   z2 = relu(psum2 + b2)  -> bf16                 (per-partition bias)
  L3: 16 matmuls psum3[b, cols] += z2_slice^T @ vb_p            (K=128, M=128, N=5..10)
  softmax: exp (ACT), grouped reduce_sum + reciprocal (DVE), multiply (GpSimd)
  DMA: batched per macro - xw/wb loads on the SP HWDGE ring, the [128, 160]
  f32 output store on the GpSimd SWDGE queue; host strips padding + unsorts.

The program is compiled per call for the actual supertile count
(rounded to whole macros) rather than the worst-case T=20; with the
reference inputs that is 18 slots/core (9 macros).
Measured on 8 axon NeuronCores: ~60.5 us HW exec, absmax err ~1.1e-3.
"""

import numpy as np
import ml_dtypes

B, J, Q, O = 32768, 32, 7, 5
H = 64            # H1 == H2
ST = 256          # samples per supertile
T = 20            # supertiles per core (worst case 32768/256 + 32 = 160 = 8*20)
N_CORES = 8
QP = 4            # question pairs (Q=7 padded to 8)
XW = QP * ST + 672  # merged xt+wa row length (wa 592 + partner b3 80)
XWM = 1152          # per-group payload length in the 3-chunk macro layout
XWS = 1184          # chunk stride (pad breaks AP coalescing -> 36 DMA descriptors)
WBW = 300           # per-tile wb width (172 + mixed-W2 block 128)

_bf16 = ml_dtypes.bfloat16
_cache = {}


# ----------------------------------------------------------------------------
# device program
# ----------------------------------------------------------------------------

def _build_program(teff):
    import concourse.bacc as bacc
    import concourse.tile as tile
    import concourse.mybir as mybir
    import concourse.bass as bass
    from contextlib import ExitStack

    bf = mybir.dt.bfloat16
    f32 = mybir.dt.float32
    AF = mybir.ActivationFunctionType

    nc = bacc.Bacc("TRN2", target_bir_lowering=False, debug=False)
    # xw: per-macro, 12 partitions x 3 chunks (stride XWS, payload XWM) so
    # the DMA emits 36 descriptors (vs 12) and engages all 16 SDMA engines.
    # chunk 0: u0 pairs 0,1 xt | wa | b3both row at cols 768..928
    # chunk 1: u0 pairs 2,3(q6) xt | wa
    # chunk 2: u1 pairs 0..2 xt (cols 0..768) | wa (768..1152)
    xw_d = nc.dram_tensor("xw", (teff // 2, 12, 3, XWM), bf, kind="ExternalInput")
    wb_d = nc.dram_tensor("wb", (teff // 2, 128, 472), bf, kind="ExternalInput")
    out_d = nc.dram_tensor("out", (teff, 128, 80), f32, kind="ExternalOutput")

    def bcast_last(ap, n):
        return bass.AP(ap.tensor, ap.offset, list(ap.ap) + [[0, n]])

    with ExitStack() as ctx:
        tc = ctx.enter_context(tile.TileContext(nc))
        cpool = ctx.enter_context(tc.tile_pool(name="const", bufs=1))
        inp = ctx.enter_context(tc.tile_pool(name="inp", bufs=6))
        zpool = ctx.enter_context(tc.tile_pool(name="z", bufs=2))
        spool = ctx.enter_context(tc.tile_pool(name="soft", bufs=2))
        pp1 = ctx.enter_context(tc.tile_pool(name="pp1", bufs=2, space="PSUM"))
        pp2 = ctx.enter_context(tc.tile_pool(name="pp2", bufs=1, space="PSUM"))
        pp3 = ctx.enter_context(tc.tile_pool(name="pp3", bufs=2, space="PSUM"))

        ones_col = cpool.tile([1, 128], bf)
        nc.vector.memset(ones_col[:], 1.0)


        # two supertiles per macro-iteration: DMAs and softmax tail batched
        for m in range(teff // 2):
            xw = inp.tile([12, 3 * XWS], bf, tag="xw")
            xw_dst = bass.AP(xw[:].tensor, xw[:].offset,
                             [list(xw[:].ap[0]), [XWS, 3], [1, XWM]])
            nc.sync.dma_start(xw_dst, xw_d.ap()[m])
            wb = inp.tile([128, 472], bf, tag="wb")
            nc.sync.dma_start(wb[:], wb_d.ap()[m])

            exps = spool.tile([128, 160], f32, tag="exps")
            p3 = pp3.tile([128, 160], f32, tag="p3")
            # one bias matmul for both sub-tiles: rhs gathers the two b3 rows
            nc.tensor.matmul(p3[:], ones_col[:], xw[0:1, 768:928],
                             start=True, stop=False)
            z2s = []
            for u in range(2):
                # sub-tile u0 also carries the shared q6 block (s0 = u0's q6,
                # s1 = u1's q6) as its 4th pair; u1 has only 3 pairs.
                npair = 4 if u == 0 else 3
                width = npair * ST
                wbo = u * 300
                # L1
                p1 = pp1.tile([128, width], f32, tag="p1")
                for p in range(npair):
                    if u == 0:
                        cb, lp = XWS * (p // 2), p % 2
                        lhs = xw[:, cb + 512 + lp * 128: cb + 512 + (lp + 1) * 128]
                        rhs = xw[:, cb + lp * ST: cb + (lp + 1) * ST]
                    else:
                        lhs = xw[:, 2 * XWS + 768 + p * 128:
                                 2 * XWS + 768 + (p + 1) * 128]
                        rhs = xw[:, 2 * XWS + p * ST: 2 * XWS + (p + 1) * ST]
                    nc.tensor.matmul(
                        p1[:, p * ST:(p + 1) * ST], lhs, rhs,
                        start=True, stop=True)
                z1 = zpool.tile([128, width], bf, tag="z1")
                nc.vector.tensor_scalar_max(z1[:], p1[:], 0.0)

                # L2: pairs 0-2 share one stationary -> N=512 + N=256
                p2 = pp2.tile([128, width], f32, tag="p2")
                nc.tensor.matmul(
                    p2[:, 0:512], wb[:, wbo:wbo + 128], z1[:, 0:512],
                    start=True, stop=True)
                nc.tensor.matmul(
                    p2[:, 512:768], wb[:, wbo:wbo + 128], z1[:, 512:768],
                    start=True, stop=True)
                if u == 0:
                    # q6 stationary: dedicated [[W2(j0),0],[0,W2(j1)]] block
                    nc.tensor.matmul(
                        p2[:, 3 * ST:4 * ST], wb[:, 172:300],
                        z1[:, 3 * ST:4 * ST],
                        start=True, stop=True)
                z2 = zpool.tile([128, width], bf, tag="z2")
                b2ap = wb[:, wbo + 168:wbo + 170].bitcast(f32)
                if u == 0:
                    # q6 block needs the mixed bias [b2(j0); b2(j1)]
                    nc.scalar.activation(z2[:, :3 * ST], p2[:, :3 * ST],
                                         AF.Relu, bias=b2ap, scale=1.0)
                    b2mix = wb[:, 170:172].bitcast(f32)
                    nc.scalar.activation(z2[:, 3 * ST:], p2[:, 3 * ST:],
                                         AF.Relu, bias=b2mix, scale=1.0)
                else:
                    nc.scalar.activation(z2[:], p2[:], AF.Relu, bias=b2ap,
                                         scale=1.0)
                z2s.append(z2)

                # L3 per-(pair, block) products accumulate onto the bias
                for p in range(3):
                    for blk in range(2):
                        nc.tensor.matmul(
                            p3[:, u * 80 + blk * 40 + p * 10:
                               u * 80 + blk * 40 + p * 10 + 10],
                            z2[:, p * ST + blk * 128: p * ST + blk * 128 + 128],
                            wb[:, wbo + 128 + p * 10: wbo + 128 + p * 10 + 10],
                            start=False, stop=False)
            # shared q6 products: one MM per blk, two-range out AP lands
            # s=0 cols in u0's q6 slot and s=1 cols in u1's
            for blk in range(2):
                o3 = p3[:, blk * 40 + 30: blk * 40 + 35]
                out2 = bass.AP(o3.tensor, o3.offset,
                               [list(o3.ap[0]), [80, 2], [1, 5]])
                nc.tensor.matmul(
                    out2,
                    z2s[0][:, 3 * ST + blk * 128: 3 * ST + blk * 128 + 128],
                    wb[:, 128 + 30: 128 + 40],
                    start=False, stop=(blk == 1))
            nc.scalar.activation(exps[:], p3[:], AF.Exp)

            # softmax tail over both supertiles at once
            den = spool.tile([128, 32], f32, tag="den")
            nc.vector.reduce_sum(
                den[:], exps[:].rearrange("p (g o) -> p g o", o=5),
                axis=mybir.AxisListType.X)
            rden = spool.tile([128, 32], f32, tag="rden")
            nc.vector.reciprocal_approx_fast(rden[:], den[:])
            probs = spool.tile([128, 160], f32, tag="probs")
            nc.gpsimd.tensor_tensor(
                probs[:].rearrange("p (g o) -> p g o", o=5),
                exps[:].rearrange("p (g o) -> p g o", o=5),
                bcast_last(rden[:], 5),
                mybir.AluOpType.mult)

            # last store goes on the SP HWDGE ring: no loads follow it, and
            # it spares the exit drain the slow SWDGE completion wait
            eng = nc.sync if m == teff // 2 - 1 else nc.gpsimd
            eng.dma_start(
                out_d.ap()[2 * m:2 * m + 2].rearrange("t p kc -> p t kc"),
                probs[:].rearrange("p (t kc) -> p t kc", t=2))

    nc.compile()
    return nc


def _get_program(teff=T):
    if teff not in _cache:
        _cache[teff] = _build_program(teff)
    return _cache[teff]


# ----------------------------------------------------------------------------
# host-side data prep
# ----------------------------------------------------------------------------

def _expert_blobs(W1, W1_a, W2, W2_a, V, V_a):
    """Per-expert wa [J,64,128] and wb [J,128,172] arrays (uint16 bf16 bits)."""
    W1c = (W1[None] + W1_a).astype(np.float32)    # [J,Q,H,O+1]
    W2c = (W2[None] + W2_a).astype(np.float32)    # [J,H,H+1]
    Vc = (V[None] + V_a).astype(np.float32)       # [J,Q,O,H+1]

    wa = np.zeros((J, 12, 592), np.float32)
    for q in range(Q):
        p, s = q // 2, q % 2
        # [J, d, h] <- W1c[:, q] is [J, h, d]
        wa[:, s * 6:s * 6 + 6, p * 128 + s * 64: p * 128 + (s + 1) * 64] = \
            W1c[:, q].transpose(0, 2, 1)
        for blk in range(2):
            wa[:, 0, 512 + blk * 40 + p * 10 + s * 5:
               512 + blk * 40 + p * 10 + s * 5 + 5] = Vc[:, q, :, 0]
    wa16 = wa.astype(_bf16).view(np.uint16)   # goes into xw cols QP*ST..XW

    wb = np.zeros((J, 128, 168), np.float32)
    w2w = W2c[:, :, 1:].transpose(0, 2, 1)        # [J, i, h2]
    for s in range(2):
        wb[:, s * 64:(s + 1) * 64, s * 64:(s + 1) * 64] = w2w
    for q in range(Q):
        p, s = q // 2, q % 2
        # [J, h2, o] <- Vc[:, q, :, 1:] is [J, o, h2]
        wb[:, s * 64:(s + 1) * 64, 128 + p * 10 + s * 5: 128 + p * 10 + s * 5 + 5] = \
            Vc[:, q, :, 1:].transpose(0, 2, 1)
    wb16 = np.zeros((J, 128, WBW), np.uint16)
    wb16[:, :, :168] = wb.astype(_bf16).view(np.uint16)
    wb16[:, :, 172:300] = wb16[:, :, 0:128]
    b2 = np.concatenate([W2c[:, :, 0], W2c[:, :, 0]], axis=1)  # [J, 128]
    wb16[:, :, 168:170] = b2.astype(np.float32).view(np.uint16).reshape(J, 128, 2)
    # per-macro q6 patch blocks (partner expert's q6 weights, s=1 halves)
    q6w1 = W1c[:, 6].transpose(0, 2, 1).astype(_bf16).view(np.uint16)     # [J,6,64]
    q6v = Vc[:, 6, :, 1:].transpose(0, 2, 1).astype(_bf16).view(np.uint16)  # [J,64,5]
    return wa16, wb16, q6w1, q6v


def _plan(judge_ids):
    """Supertile schedule: list of (judge, sample_idx_array), core/slot map."""
    jid = np.asarray(judge_ids).astype(np.int64).ravel()
    assert jid.shape[0] == B
    order = np.argsort(jid, kind="stable")
    counts = np.bincount(jid, minlength=J)
    tiles = []
    pos = 0
    for j in range(J):
        g = order[pos:pos + counts[j]]
        pos += counts[j]
        for s in range(0, len(g), ST):
            tiles.append((j, g[s:s + ST]))
    assert len(tiles) <= N_CORES * T, f"{len(tiles)} supertiles > capacity"
    return tiles


def _prepare_inputs(x, judge_ids, W1, W1_a, W2, W2_a, V, V_a):
    x = np.ascontiguousarray(np.asarray(x, dtype=np.float32))
    wa16, wb16, q6w1, q6v = _expert_blobs(*(np.asarray(a, dtype=np.float32)
                                            for a in (W1, W1_a, W2, W2_a, V, V_a)))
    tiles = _plan(judge_ids)
    # compile/run for the actual slot count (rounded to whole macros),
    # not the worst-case T
    teff = -(-len(tiles) // N_CORES)
    teff += teff % 2

    judge_mat = np.zeros((N_CORES, teff), np.int64)      # expert per slot
    xg = np.zeros((N_CORES, teff, ST, Q, O), np.float32)  # gathered x
    for i, (j, g) in enumerate(tiles):
        k, t = i % N_CORES, i // N_CORES
        judge_mat[k, t] = j
        xg[k, t, :len(g)] = x[g]

    xt = np.zeros((N_CORES, teff, 12, QP * ST), np.float32)
    xt[:, :, 0, :] = 1.0
    xt[:, :, 6, :] = 1.0
    for q in range(Q):
        p, s = q // 2, q % 2
        xt[:, :, s * 6 + 1:s * 6 + 6, p * ST:(p + 1) * ST] = \
            xg[:, :, :, q, :].transpose(0, 1, 3, 2)
    # shared q6 block: even (u0) slots carry the partner's q6 on the s=1 rows
    xt[:, 0::2, 7:12, 3 * ST:4 * ST] = \
        xg[:, 1::2, :, 6, :].transpose(0, 1, 3, 2)
    xt16 = xt.astype(_bf16).view(np.uint16)

    j1 = judge_mat[:, 1::2]   # partner expert of each macro
    in_maps = []
    for k in range(N_CORES):
        was = np.zeros((teff, 12, 672), np.uint16)
        was[:, :, :592] = wa16[judge_mat[k]]
        # partner's q6 L1 block into the s=1 half of u0's 4th pair
        was[0::2, 6:12, 448:512] = q6w1[j1[k]]
        # partner's b3 row appended after own b3 (one contiguous 160-col rhs)
        was[0::2, 0, 592:672] = was[1::2, 0, 512:592]
        xwm = np.zeros((teff // 2, 12, 3, XWM), np.uint16)
        xwm[:, :, 0, 0:512] = xt16[k, 0::2, :, 0:512]
        xwm[:, :, 0, 512:768] = was[0::2, :, 0:256]
        xwm[:, 0, 0, 768:928] = was[0::2, 0, 512:672]
        xwm[:, :, 1, 0:512] = xt16[k, 0::2, :, 512:1024]
        xwm[:, :, 1, 512:768] = was[0::2, :, 256:512]
        xwm[:, :, 2, 0:768] = xt16[k, 1::2, :, 0:768]
        xwm[:, :, 2, 768:1152] = was[1::2, :, 0:384]
        wbs = wb16[judge_mat[k]].copy()
        # partner's q6 V block into the s=1 half of u0's p3 slot
        wbs[0::2, 64:128, 163:168] = q6v[j1[k]]
        # mixed L2 bias [b2(j0); b2(j1)] for the q6 block, f32 bits
        wbs[0::2, 64:128, 170:172] = wbs[1::2, 64:128, 168:170]
        wbs[0::2, 0:64, 170:172] = wbs[0::2, 0:64, 168:170]
        # partner's W2 into the s=1 half of the mixed block
        wbs[0::2, 64:128, 236:300] = wbs[1::2, 64:128, 64:128]
        wbm = np.zeros((teff // 2, 128, 472), np.uint16)
        wbm[:, :, 0:300] = wbs[0::2]
        wbm[:, :, 300:472] = wbs[1::2, :, 0:172]
        in_maps.append({
            "xw": np.ascontiguousarray(xwm).view(_bf16),
            "wb": np.ascontiguousarray(wbm).view(_bf16),
        })
    return in_maps, tiles, teff


def _assemble_output(results, tiles):
    out = np.empty((B, Q, O), np.float32)
    for i, (_, g) in enumerate(tiles):
        k, t = i % N_CORES, i // N_CORES
        blob = results[k]["out"][t].reshape(128, 2, 40)
        rows = blob.transpose(1, 0, 2).reshape(ST, 40)[:len(g), :35]
        out[g] = rows.reshape(len(g), Q, O)
    return out


# ----------------------------------------------------------------------------
# entry point
# ----------------------------------------------------------------------------

def kernel(x, judge_ids, W1, W1_a, W2, W2_a, V, V_a):
    from concourse import bass_utils
    in_maps, tiles, teff = _prepare_inputs(x, judge_ids, W1, W1_a, W2, W2_a, V, V_a)
    nc = _get_program(teff)
    res = bass_utils.run_bass_kernel_spmd(
        nc, in_maps, core_ids=list(range(N_CORES)), trace=False)
    return _assemble_output(res.results, tiles)


# expose for test harness reuse
def run_with_results(x, judge_ids, W1, W1_a, W2, W2_a, V, V_a, trace=False,
                     **kwargs):
    from concourse import bass_utils
    in_maps, tiles, teff = _prepare_inputs(x, judge_ids, W1, W1_a, W2, W2_a, V, V_a)
    nc = _get_program(teff)
    res = bass_utils.run_bass_kernel_spmd(
        nc, in_maps, core_ids=list(range(N_CORES)), trace=trace, **kwargs)
    return _assemble_output(res.results, tiles), res



# revision 13
# speedup vs baseline: 1.0090x; 1.0090x over previous
"""Trainium2 Bass kernel for nn_CalibrationNetwork (MoE-routed 3-layer MLP + softmax).

Strategy
--------
Host side (numpy): sort samples by judge id, pad each judge group to a
multiple of 256 samples ("supertile"), distribute supertiles round-robin
over 8 cores (20 slots/core covers the worst case sum_j ceil(c_j/256) <= 160).
All judge-specific tables are combined with the shared ones on the host
(W1+W1_a[j] etc.), so the device never gathers. Questions are processed as
block-diagonal pairs (two 64-wide blocks fill the 128 partitions/PE columns);
the odd 7th question (q6) of the two supertiles in a macro-iteration shares
one block (s=0 half = first tile's q6, s=1 half = second tile's q6, with the
two tiles' experts mixed in the block-diagonal weights).

Per supertile the host emits, merged into two HBM streams:
  - xw [12, XW] bf16: cols 0..1024 = transposed x with bias-ones rows (rows
    s*6+d; d=0 is ones), pair p in cols p*256; cols 1024..1616 = L1
    stationary blocks (rows s*6+d, cols p*128+s*64+h) with partition-0 cols
    1536..1696 holding both tiles' L3 bias rows b3 (o-major, (blk,p,s,o)).
  - wb [128, 300] bf16: cols 0..128 L2 block-diag stationary [[W2,0],[0,W2]];
    cols 128..168 L3 moving operand (V block per pair, cols p*10+s*5+o);
    cols 168..172 the L2 bias columns as raw f32 bits (bitcast on device,
    own bias then q6-mixed bias); cols 172..300 the q6-mixed L2 stationary
    [[W2(j0),0],[0,W2(j1)]].

Device (per macro = two supertiles; all engines pipelined by Tile):
  bias: 1 matmul ones_col^T @ b3rows -> psum3 [128, 160]        (K=1, N=160)
  L1: 7 matmuls  psum1[(s,h1), (p,b)] += wa_p^T @ xt_p          (K=12, M=128, N=256)
  relu1 (DVE):   z1 = max(psum1, 0) -> bf16                     (bias via ones row)
  L2: 7 matmuls  psum2[(s,h2), (p,b)] += w2blk^T @ z1_p         (K=128, M=128, N=256)
  relu2 (ACT):   z2 = relu(psum2 + b2)  -> bf16                 (per-partition bias)
  L3: 16 matmuls psum3[b, cols] += z2_slice^T @ vb_p            (K=128, M=128, N=5..10)
  softmax: exp (ACT), grouped reduce_sum + reciprocal (DVE), multiply (GpSimd)
  DMA: batched per macro - xw/wb loads on the SP HWDGE ring, the [128, 160]
  f32 output store on the GpSimd SWDGE queue; host strips padding + unsorts.

The program is compiled per call for the actual supertile count
(rounded to whole macros) rather than the worst-case T=20; with the
reference inputs that is 18 slots/core (9 macros).
Measured on 8 axon NeuronCores: ~60.5 us HW exec, absmax err ~1.1e-3.
"""

import numpy as np
import ml_dtypes

B, J, Q, O = 32768, 32, 7, 5
H = 64            # H1 == H2
ST = 256          # samples per supertile
T = 20            # supertiles per core (worst case 32768/256 + 32 = 160 = 8*20)
N_CORES = 8
QP = 4            # question pairs (Q=7 padded to 8)
XWW = 928           # xw row length: xt_u0 256 | xt_u1 256 | wa_u0 128 | wa_u1 128 | b3 160
WBW = 300           # per-tile wb width (172 + mixed-W2 block 128)

_bf16 = ml_dtypes.bfloat16
_cache = {}


# ----------------------------------------------------------------------------
# device program
# ----------------------------------------------------------------------------

def _build_program(teff):
    import concourse.bacc as bacc
    import concourse.tile as tile
    import concourse.mybir as mybir
    import concourse.bass as bass
    from contextlib import ExitStack

    bf = mybir.dt.bfloat16
    f32 = mybir.dt.float32
    AF = mybir.ActivationFunctionType

    nc = bacc.Bacc("TRN2", target_bir_lowering=False, debug=False)
    # xw: per-macro [108, XWW].  Row group g (partitions 32g..32g+11) holds
    # question-pair g so the 7 L1 matmuls run 4-way concurrent via PE row
    # tiling (tile_position auto-derives from the 32-aligned operand bases).
    # Per group: cols 0..256 xt_u0 | 256..512 xt_u1 | 512..640 wa_u0 |
    # 640..768 wa_u1; partition 32 cols 768..928 holds both tiles' b3 rows.
    # Rows 12..31 of each group are zero padding (DMA has headroom).
    xw_d = nc.dram_tensor("xw", (teff // 2, 108, XWW), bf, kind="ExternalInput")
    wb_d = nc.dram_tensor("wb", (teff // 2, 128, 472), bf, kind="ExternalInput")
    out_d = nc.dram_tensor("out", (teff, 128, 80), f32, kind="ExternalOutput")

    def bcast_last(ap, n):
        return bass.AP(ap.tensor, ap.offset, list(ap.ap) + [[0, n]])

    with ExitStack() as ctx:
        tc = ctx.enter_context(tile.TileContext(nc))
        cpool = ctx.enter_context(tc.tile_pool(name="const", bufs=1))
        inp = ctx.enter_context(tc.tile_pool(name="inp", bufs=6))
        zpool = ctx.enter_context(tc.tile_pool(name="z", bufs=2))
        spool = ctx.enter_context(tc.tile_pool(name="soft", bufs=2))
        pp1 = ctx.enter_context(tc.tile_pool(name="pp1", bufs=1, space="PSUM"))
        pp2 = ctx.enter_context(tc.tile_pool(name="pp2", bufs=1, space="PSUM"))
        pp3 = ctx.enter_context(tc.tile_pool(name="pp3", bufs=2, space="PSUM"))

        ones_col = cpool.tile([128, 128], bf)
        nc.vector.memset(ones_col[:], 1.0)


        # two supertiles per macro-iteration: DMAs and softmax tail batched
        for m in range(teff // 2):
            xw = inp.tile([128, XWW], bf, tag="xw")
            nc.sync.dma_start(xw[0:108, :], xw_d.ap()[m])
            wb = inp.tile([128, 472], bf, tag="wb")
            nc.sync.dma_start(wb[:], wb_d.ap()[m])

            exps = spool.tile([128, 160], f32, tag="exps")
            p3 = pp3.tile([128, 160], f32, tag="p3")
            # L1: row group g = pair g; u0's 4 matmuls run concurrently on
            # the 4 PE row tiles (one PSUM bank each), u1's 3 pipeline behind.
            p1 = pp1.tile([128, 2048], f32, tag="p1")
            for u in range(2):
                for g in range(4 if u == 0 else 3):
                    nc.tensor.matmul(
                        p1[:, g * 512 + u * ST: g * 512 + u * ST + ST],
                        xw[32 * g: 32 * g + 12, 512 + u * 128: 640 + u * 128],
                        xw[32 * g: 32 * g + 12, u * ST: u * ST + ST],
                        start=True, stop=True, tile_position=(32 * g, 0))
            # one bias matmul for both sub-tiles (also 32-row tile mode, so
            # no PE mode switch until L2): rhs holds the two b3 rows
            nc.tensor.matmul(
                p3[:], ones_col[32:33, :],
                xw[32:33, 768:928], start=True, stop=False,
                tile_position=(32, 0))
            z1s = []
            for u in range(2):
                npair = 4 if u == 0 else 3
                width = npair * ST
                base = p1[:, u * ST: u * ST + 1]
                src = bass.AP(base.tensor, base.offset,
                              [list(base.ap[0]), [512, npair], [1, ST]])
                z1 = zpool.tile([128, width], bf, tag="z1")
                nc.vector.tensor_scalar_max(z1[:], src, 0.0)
                z1s.append(z1)
            z2s = []
            for u in range(2):
                # sub-tile u0 also carries the shared q6 block (s0 = u0's q6,
                # s1 = u1's q6) as its 4th pair; u1 has only 3 pairs.
                npair = 4 if u == 0 else 3
                width = npair * ST
                wbo = u * 300
                z1 = z1s[u]

                # L2: pairs 0-2 share one stationary -> N=512 + N=256
                p2 = pp2.tile([128, width], f32, tag="p2")
                nc.tensor.matmul(
                    p2[:, 0:512], wb[:, wbo:wbo + 128], z1[:, 0:512],
                    start=True, stop=True)
                nc.tensor.matmul(
                    p2[:, 512:768], wb[:, wbo:wbo + 128], z1[:, 512:768],
                    start=True, stop=True)
                if u == 0:
                    # q6 stationary: dedicated [[W2(j0),0],[0,W2(j1)]] block
                    nc.tensor.matmul(
                        p2[:, 3 * ST:4 * ST], wb[:, 172:300],
                        z1[:, 3 * ST:4 * ST],
                        start=True, stop=True)
                z2 = zpool.tile([128, width], bf, tag="z2")
                b2ap = wb[:, wbo + 168:wbo + 170].bitcast(f32)
                if u == 0:
                    # q6 block needs the mixed bias [b2(j0); b2(j1)]
                    nc.scalar.activation(z2[:, :3 * ST], p2[:, :3 * ST],
                                         AF.Relu, bias=b2ap, scale=1.0)
                    b2mix = wb[:, 170:172].bitcast(f32)
                    nc.scalar.activation(z2[:, 3 * ST:], p2[:, 3 * ST:],
                                         AF.Relu, bias=b2mix, scale=1.0)
                else:
                    nc.scalar.activation(z2[:], p2[:], AF.Relu, bias=b2ap,
                                         scale=1.0)
                z2s.append(z2)

                # L3 per-(pair, block) products accumulate onto the bias
                for p in range(3):
                    for blk in range(2):
                        nc.tensor.matmul(
                            p3[:, u * 80 + blk * 40 + p * 10:
                               u * 80 + blk * 40 + p * 10 + 10],
                            z2[:, p * ST + blk * 128: p * ST + blk * 128 + 128],
                            wb[:, wbo + 128 + p * 10: wbo + 128 + p * 10 + 10],
                            start=False, stop=False)
            # shared q6 products: one MM per blk, two-range out AP lands
            # s=0 cols in u0's q6 slot and s=1 cols in u1's
            for blk in range(2):
                o3 = p3[:, blk * 40 + 30: blk * 40 + 35]
                out2 = bass.AP(o3.tensor, o3.offset,
                               [list(o3.ap[0]), [80, 2], [1, 5]])
                nc.tensor.matmul(
                    out2,
                    z2s[0][:, 3 * ST + blk * 128: 3 * ST + blk * 128 + 128],
                    wb[:, 128 + 30: 128 + 40],
                    start=False, stop=(blk == 1))
            nc.scalar.activation(exps[:], p3[:], AF.Exp)

            # softmax tail over both supertiles at once
            den = spool.tile([128, 32], f32, tag="den")
            nc.vector.reduce_sum(
                den[:], exps[:].rearrange("p (g o) -> p g o", o=5),
                axis=mybir.AxisListType.X)
            rden = spool.tile([128, 32], f32, tag="rden")
            nc.vector.reciprocal_approx_fast(rden[:], den[:])
            probs = spool.tile([128, 160], f32, tag="probs")
            nc.gpsimd.tensor_tensor(
                probs[:].rearrange("p (g o) -> p g o", o=5),
                exps[:].rearrange("p (g o) -> p g o", o=5),
                bcast_last(rden[:], 5),
                mybir.AluOpType.mult)

            # last store goes on the SP HWDGE ring: no loads follow it, and
            # it spares the exit drain the slow SWDGE completion wait
            eng = nc.sync if m == teff // 2 - 1 else nc.gpsimd
            eng.dma_start(
                out_d.ap()[2 * m:2 * m + 2].rearrange("t p kc -> p t kc"),
                probs[:].rearrange("p (t kc) -> p t kc", t=2))

    nc.compile()
    return nc


def _get_program(teff=T):
    if teff not in _cache:
        _cache[teff] = _build_program(teff)
    return _cache[teff]


# ----------------------------------------------------------------------------
# host-side data prep
# ----------------------------------------------------------------------------

def _expert_blobs(W1, W1_a, W2, W2_a, V, V_a):
    """Per-expert weight blobs (uint16 bf16 bits)."""
    W1c = (W1[None] + W1_a).astype(np.float32)    # [J,Q,H,O+1]
    W2c = (W2[None] + W2_a).astype(np.float32)    # [J,H,H+1]
    Vc = (V[None] + V_a).astype(np.float32)       # [J,Q,O,H+1]

    # waq[j, p, 6s+d, s*64+h] = W1c[j, 2p+s, h, d]: L1 stationary per pair
    wa = np.zeros((J, 3, 12, 128), np.float32)
    for q in range(Q - 1):
        p, s = q // 2, q % 2
        wa[:, p, s * 6:s * 6 + 6, s * 64:(s + 1) * 64] = \
            W1c[:, q].transpose(0, 2, 1)
    # b3 row per expert: col blk*40 + p*10 + s*5 + o = Vc[j, 2p+s, o, 0]
    b3 = np.zeros((J, 80), np.float32)
    for q in range(Q):
        p, s = q // 2, q % 2
        for blk in range(2):
            b3[:, blk * 40 + p * 10 + s * 5: blk * 40 + p * 10 + s * 5 + 5] = \
                Vc[:, q, :, 0]
    wa16 = wa.astype(_bf16).view(np.uint16)
    b316 = b3.astype(_bf16).view(np.uint16)

    wb = np.zeros((J, 128, 168), np.float32)
    w2w = W2c[:, :, 1:].transpose(0, 2, 1)        # [J, i, h2]
    for s in range(2):
        wb[:, s * 64:(s + 1) * 64, s * 64:(s + 1) * 64] = w2w
    for q in range(Q):
        p, s = q // 2, q % 2
        # [J, h2, o] <- Vc[:, q, :, 1:] is [J, o, h2]
        wb[:, s * 64:(s + 1) * 64, 128 + p * 10 + s * 5: 128 + p * 10 + s * 5 + 5] = \
            Vc[:, q, :, 1:].transpose(0, 2, 1)
    wb16 = np.zeros((J, 128, WBW), np.uint16)
    wb16[:, :, :168] = wb.astype(_bf16).view(np.uint16)
    wb16[:, :, 172:300] = wb16[:, :, 0:128]
    b2 = np.concatenate([W2c[:, :, 0], W2c[:, :, 0]], axis=1)  # [J, 128]
    wb16[:, :, 168:170] = b2.astype(np.float32).view(np.uint16).reshape(J, 128, 2)
    # per-macro q6 patch blocks (partner expert's q6 weights, s=1 halves)
    q6w1 = W1c[:, 6].transpose(0, 2, 1).astype(_bf16).view(np.uint16)     # [J,6,64]
    q6v = Vc[:, 6, :, 1:].transpose(0, 2, 1).astype(_bf16).view(np.uint16)  # [J,64,5]
    return wa16, b316, wb16, q6w1, q6v


def _plan(judge_ids):
    """Supertile schedule: list of (judge, sample_idx_array), core/slot map."""
    jid = np.asarray(judge_ids).astype(np.int64).ravel()
    assert jid.shape[0] == B
    order = np.argsort(jid, kind="stable")
    counts = np.bincount(jid, minlength=J)
    tiles = []
    pos = 0
    for j in range(J):
        g = order[pos:pos + counts[j]]
        pos += counts[j]
        for s in range(0, len(g), ST):
            tiles.append((j, g[s:s + ST]))
    assert len(tiles) <= N_CORES * T, f"{len(tiles)} supertiles > capacity"
    return tiles


def _prepare_inputs(x, judge_ids, W1, W1_a, W2, W2_a, V, V_a):
    x = np.ascontiguousarray(np.asarray(x, dtype=np.float32))
    wa16, b316, wb16, q6w1, q6v = _expert_blobs(
        *(np.asarray(a, dtype=np.float32)
          for a in (W1, W1_a, W2, W2_a, V, V_a)))
    tiles = _plan(judge_ids)
    # compile/run for the actual slot count (rounded to whole macros),
    # not the worst-case T
    teff = -(-len(tiles) // N_CORES)
    teff += teff % 2

    judge_mat = np.zeros((N_CORES, teff), np.int64)      # expert per slot
    xg = np.zeros((N_CORES, teff, ST, Q, O), np.float32)  # gathered x
    for i, (j, g) in enumerate(tiles):
        k, t = i % N_CORES, i // N_CORES
        judge_mat[k, t] = j
        xg[k, t, :len(g)] = x[g]

    # xw macro blob [108, XWW]: row group g = partitions 32g..32g+11, one
    # question pair each; group 3 is the shared q6 block (s0 = u0's q6,
    # s1 = u1's q6)
    nm = teff // 2
    xwf = np.zeros((N_CORES, nm, 108, XWW), np.float32)
    for g in range(3):
        for s in range(2):
            r, q = 32 * g + 6 * s, 2 * g + s
            xwf[:, :, r, 0:512] = 1.0
            xwf[:, :, r + 1:r + 6, 0:256] = \
                xg[:, 0::2, :, q, :].transpose(0, 1, 3, 2)
            xwf[:, :, r + 1:r + 6, 256:512] = \
                xg[:, 1::2, :, q, :].transpose(0, 1, 3, 2)
    xwf[:, :, 96, 0:256] = 1.0
    xwf[:, :, 97:102, 0:256] = xg[:, 0::2, :, 6, :].transpose(0, 1, 3, 2)
    xwf[:, :, 102, 0:256] = 1.0
    xwf[:, :, 103:108, 0:256] = xg[:, 1::2, :, 6, :].transpose(0, 1, 3, 2)
    xw16 = xwf.astype(_bf16).view(np.uint16)

    j0, j1 = judge_mat[:, 0::2], judge_mat[:, 1::2]  # experts per macro
    for g in range(3):
        xw16[:, :, 32 * g:32 * g + 12, 512:640] = wa16[j0][:, :, g]
        xw16[:, :, 32 * g:32 * g + 12, 640:768] = wa16[j1][:, :, g]
    # q6 mixed L1 stationary in group 3
    xw16[:, :, 96:102, 512:576] = q6w1[j0]
    xw16[:, :, 102:108, 576:640] = q6w1[j1]
    # both tiles' b3 rows on partition 32
    xw16[:, :, 32, 768:848] = b316[j0]
    xw16[:, :, 32, 848:928] = b316[j1]

    in_maps = []
    for k in range(N_CORES):
        wbs = wb16[judge_mat[k]].copy()
        # partner's q6 V block into the s=1 half of u0's p3 slot
        wbs[0::2, 64:128, 163:168] = q6v[j1[k]]
        # mixed L2 bias [b2(j0); b2(j1)] for the q6 block, f32 bits
        wbs[0::2, 64:128, 170:172] = wbs[1::2, 64:128, 168:170]
        wbs[0::2, 0:64, 170:172] = wbs[0::2, 0:64, 168:170]
        # partner's W2 into the s=1 half of the mixed block
        wbs[0::2, 64:128, 236:300] = wbs[1::2, 64:128, 64:128]
        wbm = np.zeros((teff // 2, 128, 472), np.uint16)
        wbm[:, :, 0:300] = wbs[0::2]
        wbm[:, :, 300:472] = wbs[1::2, :, 0:172]
        in_maps.append({
            "xw": np.ascontiguousarray(xw16[k]).view(_bf16),
            "wb": np.ascontiguousarray(wbm).view(_bf16),
        })
    return in_maps, tiles, teff


def _assemble_output(results, tiles):
    out = np.empty((B, Q, O), np.float32)
    for i, (_, g) in enumerate(tiles):
        k, t = i % N_CORES, i // N_CORES
        blob = results[k]["out"][t].reshape(128, 2, 40)
        rows = blob.transpose(1, 0, 2).reshape(ST, 40)[:len(g), :35]
        out[g] = rows.reshape(len(g), Q, O)
    return out


# ----------------------------------------------------------------------------
# entry point
# ----------------------------------------------------------------------------

def kernel(x, judge_ids, W1, W1_a, W2, W2_a, V, V_a):
    from concourse import bass_utils
    in_maps, tiles, teff = _prepare_inputs(x, judge_ids, W1, W1_a, W2, W2_a, V, V_a)
    nc = _get_program(teff)
    res = bass_utils.run_bass_kernel_spmd(
        nc, in_maps, core_ids=list(range(N_CORES)), trace=False)
    return _assemble_output(res.results, tiles)


# expose for test harness reuse
def run_with_results(x, judge_ids, W1, W1_a, W2, W2_a, V, V_a, trace=False,
                     **kwargs):
    from concourse import bass_utils
    in_maps, tiles, teff = _prepare_inputs(x, judge_ids, W1, W1_a, W2, W2_a, V, V_a)
    nc = _get_program(teff)
    res = bass_utils.run_bass_kernel_spmd(
        nc, in_maps, core_ids=list(range(N_CORES)), trace=trace, **kwargs)
    return _assemble_output(res.results, tiles), res



# revision 14
# speedup vs baseline: 1.0529x; 1.0435x over previous
"""Trainium2 Bass kernel for nn_CalibrationNetwork (MoE-routed 3-layer MLP + softmax).

Strategy
--------
Host side (numpy): sort samples by judge id, pad each judge group to a
multiple of 256 samples ("supertile"), distribute supertiles round-robin
over 8 cores (20 slots/core covers the worst case sum_j ceil(c_j/256) <= 160).
All judge-specific tables are combined with the shared ones on the host
(W1+W1_a[j] etc.), so the device never gathers. Questions are processed as
block-diagonal pairs (two 64-wide blocks fill the 128 partitions/PE columns);
the odd 7th question (q6) of the two supertiles in a macro-iteration shares
one block (s=0 half = first tile's q6, s=1 half = second tile's q6, with the
two tiles' experts mixed in the block-diagonal weights).

Per supertile the host emits, merged into two HBM streams:
  - xw [12, XW] bf16: cols 0..1024 = transposed x with bias-ones rows (rows
    s*6+d; d=0 is ones), pair p in cols p*256; cols 1024..1616 = L1
    stationary blocks (rows s*6+d, cols p*128+s*64+h) with partition-0 cols
    1536..1696 holding both tiles' L3 bias rows b3 (o-major, (blk,p,s,o)).
  - wb [128, 300] bf16: cols 0..128 L2 block-diag stationary [[W2,0],[0,W2]];
    cols 128..168 L3 moving operand (V block per pair, cols p*10+s*5+o);
    cols 168..172 the L2 bias columns as raw f32 bits (bitcast on device,
    own bias then q6-mixed bias); cols 172..300 the q6-mixed L2 stationary
    [[W2(j0),0],[0,W2(j1)]].

Device (per macro = two supertiles; all engines pipelined by Tile):
  bias: 1 matmul ones_col^T @ b3rows -> psum3 [128, 160]        (K=1, N=160)
  L1: 7 matmuls  psum1[(s,h1), (p,b)] += wa_p^T @ xt_p          (K=12, M=128, N=256)
  relu1 (DVE):   z1 = max(psum1, 0) -> bf16                     (bias via ones row)
  L2: 7 matmuls  psum2[(s,h2), (p,b)] += w2blk^T @ z1_p         (K=128, M=128, N=256)
  relu2 (ACT):   z2 = relu(psum2 + b2)  -> bf16                 (per-partition bias)
  L3: 16 matmuls psum3[b, cols] += z2_slice^T @ vb_p            (K=128, M=128, N=5..10)
  softmax: exp (ACT), grouped reduce_sum + reciprocal (DVE), multiply (GpSimd)
  DMA: batched per macro - xw/wb loads on the SP HWDGE ring, the [128, 160]
  f32 output store on the GpSimd SWDGE queue; host strips padding + unsorts.

The program is compiled per call for the actual supertile count
(rounded to whole macros) rather than the worst-case T=20; with the
reference inputs that is 18 slots/core (9 macros).
Measured on 8 axon NeuronCores: ~60.5 us HW exec, absmax err ~1.1e-3.
"""

import numpy as np
import ml_dtypes

B, J, Q, O = 32768, 32, 7, 5
H = 64            # H1 == H2
ST = 256          # samples per supertile
T = 20            # supertiles per core (worst case 32768/256 + 32 = 160 = 8*20)
N_CORES = 8
QP = 4            # question pairs (Q=7 padded to 8)
XWW = 928           # xw row length: xt_u0 256 | xt_u1 256 | wa_u0 128 | wa_u1 128 | b3 160
WBW = 300           # per-tile wb width (172 + mixed-W2 block 128)

_bf16 = ml_dtypes.bfloat16
_cache = {}


# ----------------------------------------------------------------------------
# device program
# ----------------------------------------------------------------------------

def _build_program(teff):
    import concourse.bacc as bacc
    import concourse.tile as tile
    import concourse.mybir as mybir
    import concourse.bass as bass
    from contextlib import ExitStack

    bf = mybir.dt.bfloat16
    f32 = mybir.dt.float32
    AF = mybir.ActivationFunctionType

    nc = bacc.Bacc("TRN2", target_bir_lowering=False, debug=False)
    # xw: per-macro [108, XWW].  Row group g (partitions 32g..32g+11) holds
    # question-pair g so the 7 L1 matmuls run 4-way concurrent via PE row
    # tiling (tile_position auto-derives from the 32-aligned operand bases).
    # Per group: cols 0..256 xt_u0 | 256..512 xt_u1 | 512..640 wa_u0 |
    # 640..768 wa_u1; partition 32 cols 768..928 holds both tiles' b3 rows.
    # Rows 12..31 of each group are zero padding (DMA has headroom).
    xw_d = nc.dram_tensor("xw", (teff // 2, 108, XWW), bf, kind="ExternalInput")
    wb_d = nc.dram_tensor("wb", (teff // 2, 128, 472), bf, kind="ExternalInput")
    out_d = nc.dram_tensor("out", (teff, 128, 80), f32, kind="ExternalOutput")

    def bcast_last(ap, n):
        return bass.AP(ap.tensor, ap.offset, list(ap.ap) + [[0, n]])

    with ExitStack() as ctx:
        tc = ctx.enter_context(tile.TileContext(nc))
        cpool = ctx.enter_context(tc.tile_pool(name="const", bufs=1))
        inp = ctx.enter_context(tc.tile_pool(name="inp", bufs=6))
        zpool = ctx.enter_context(tc.tile_pool(name="z", bufs=2))
        spool = ctx.enter_context(tc.tile_pool(name="soft", bufs=2))
        pp1 = ctx.enter_context(tc.tile_pool(name="pp1", bufs=1, space="PSUM"))
        pp2 = ctx.enter_context(tc.tile_pool(name="pp2", bufs=1, space="PSUM"))
        pp3 = ctx.enter_context(tc.tile_pool(name="pp3", bufs=2, space="PSUM"))

        ones_col = cpool.tile([128, 128], bf)
        nc.vector.memset(ones_col[:], 1.0)


        # two supertiles per macro-iteration: DMAs and softmax tail batched
        for m in range(teff // 2):
            last = m == teff // 2 - 1
            xw = inp.tile([128, XWW], bf, tag="xw")
            nc.sync.dma_start(xw[0:108, :], xw_d.ap()[m])
            wb = inp.tile([128, 472], bf, tag="wb")
            nc.gpsimd.dma_start(wb[:], wb_d.ap()[m])

            exps = spool.tile([128, 160], f32, tag="exps")
            p3 = pp3.tile([128, 160], f32, tag="p3")
            # L1: row group g = pair g; u0's 4 matmuls run concurrently on
            # the 4 PE row tiles (one PSUM bank each), u1's 3 pipeline behind.
            p1 = pp1.tile([128, 2048], f32, tag="p1")
            for u in range(2):
                for g in range(4 if u == 0 else 3):
                    nc.tensor.matmul(
                        p1[:, g * 512 + u * ST: g * 512 + u * ST + ST],
                        xw[32 * g: 32 * g + 12, 512 + u * 128: 640 + u * 128],
                        xw[32 * g: 32 * g + 12, u * ST: u * ST + ST],
                        start=True, stop=True, tile_position=(32 * g, 0))
            # one bias matmul for both sub-tiles (also 32-row tile mode, so
            # no PE mode switch until L2): rhs holds the two b3 rows
            nc.tensor.matmul(
                p3[:], ones_col[32:33, :],
                xw[32:33, 768:928], start=True, stop=False,
                tile_position=(32, 0))
            z1s = []
            for u in range(2):
                npair = 4 if u == 0 else 3
                width = npair * ST
                base = p1[:, u * ST: u * ST + 1]
                src = bass.AP(base.tensor, base.offset,
                              [list(base.ap[0]), [512, npair], [1, ST]])
                z1 = zpool.tile([128, width], bf, tag="z1")
                nc.vector.tensor_scalar_max(z1[:], src, 0.0)
                z1s.append(z1)

            # L2 in 512-col chunks through two single-bank psum tiles so the
            # write->relu->write cycle per bank stays well under the macro
            # period (a single big p2 buffer serializes the whole pipeline)
            z1u0, z1u1 = z1s
            z2u0 = zpool.tile([128, 1024], bf, tag="z2")
            z2u1 = zpool.tile([128, 768], bf, tag="z2")
            b2u0 = wb[:, 168:170].bitcast(f32)
            b2u1 = wb[:, 468:470].bitcast(f32)
            b2mix = wb[:, 170:172].bitcast(f32)
            p2x = pp2.tile([128, 512], f32, tag="p2x")
            nc.tensor.matmul(p2x[:], wb[:, 0:128], z1u0[:, 0:512],
                             start=True, stop=True)
            p2y = pp2.tile([128, 512], f32, tag="p2y")
            nc.tensor.matmul(p2y[:, 0:256], wb[:, 0:128], z1u0[:, 512:768],
                             start=True, stop=True)
            # q6 stationary: dedicated [[W2(j0),0],[0,W2(j1)]] block
            nc.tensor.matmul(p2y[:, 256:512], wb[:, 172:300],
                             z1u0[:, 768:1024], start=True, stop=True)
            nc.scalar.activation(z2u0[:, 0:512], p2x[:], AF.Relu,
                                 bias=b2u0, scale=1.0)
            nc.scalar.activation(z2u0[:, 512:768], p2y[:, 0:256], AF.Relu,
                                 bias=b2u0, scale=1.0)
            # q6 block needs the mixed bias [b2(j0); b2(j1)]
            nc.scalar.activation(z2u0[:, 768:1024], p2y[:, 256:512], AF.Relu,
                                 bias=b2mix, scale=1.0)

            # u1 chunks reuse the banks; L3 u0 fills the PE wait on the relus
            p2x = pp2.tile([128, 512], f32, tag="p2x")
            nc.tensor.matmul(p2x[:], wb[:, 300:428], z1u1[:, 0:512],
                             start=True, stop=True)
            for p in range(2):
                for blk in range(2):
                    nc.tensor.matmul(
                        p3[:, blk * 40 + p * 10: blk * 40 + p * 10 + 10],
                        z2u0[:, p * ST + blk * 128: p * ST + blk * 128 + 128],
                        wb[:, 128 + p * 10: 128 + p * 10 + 10],
                        start=False, stop=False)
            nc.scalar.activation(z2u1[:, 0:512], p2x[:], AF.Relu,
                                 bias=b2u1, scale=1.0)
            p2y = pp2.tile([128, 512], f32, tag="p2y")
            nc.tensor.matmul(p2y[:, 0:256], wb[:, 300:428], z1u1[:, 512:768],
                             start=True, stop=True)
            for blk in range(2):
                nc.tensor.matmul(
                    p3[:, blk * 40 + 20: blk * 40 + 30],
                    z2u0[:, 2 * ST + blk * 128: 2 * ST + blk * 128 + 128],
                    wb[:, 148: 158], start=False, stop=False)
            nc.scalar.activation(z2u1[:, 512:768], p2y[:, 0:256], AF.Relu,
                                 bias=b2u1, scale=1.0)
            for p in range(3):
                for blk in range(2):
                    nc.tensor.matmul(
                        p3[:, 80 + blk * 40 + p * 10: 80 + blk * 40 + p * 10 + 10],
                        z2u1[:, p * ST + blk * 128: p * ST + blk * 128 + 128],
                        wb[:, 428 + p * 10: 428 + p * 10 + 10],
                        start=False, stop=False)
            # shared q6 products: one MM per blk, two-range out AP lands
            # s=0 cols in u0's q6 slot and s=1 cols in u1's
            for blk in range(2):
                o3 = p3[:, blk * 40 + 30: blk * 40 + 35]
                out2 = bass.AP(o3.tensor, o3.offset,
                               [list(o3.ap[0]), [80, 2], [1, 5]])
                nc.tensor.matmul(
                    out2,
                    z2u0[:, 3 * ST + blk * 128: 3 * ST + blk * 128 + 128],
                    wb[:, 128 + 30: 128 + 40],
                    start=False, stop=(blk == 1))

            # softmax tail; for the last macro run it per-supertile so the
            # first half's store (and its HBM completion wait) overlaps the
            # second half's compute
            halves = ((0, 80, nc.gpsimd), (80, 160, nc.sync)) if last \
                else ((0, 160, nc.gpsimd),)
            for lo, hi, seng in halves:
                w = hi - lo
                nc.scalar.activation(exps[:, lo:hi], p3[:, lo:hi], AF.Exp)
                den = spool.tile([128, 32], f32, tag="den")
                nc.vector.reduce_sum(
                    den[:, :w // 5],
                    exps[:, lo:hi].rearrange("p (g o) -> p g o", o=5),
                    axis=mybir.AxisListType.X)
                rden = spool.tile([128, 32], f32, tag="rden")
                nc.vector.reciprocal_approx_fast(rden[:, :w // 5],
                                                 den[:, :w // 5])
                probs = spool.tile([128, 160], f32, tag="probs")
                peng = nc.vector if last else nc.gpsimd
                peng.tensor_tensor(
                    probs[:, lo:hi].rearrange("p (g o) -> p g o", o=5),
                    exps[:, lo:hi].rearrange("p (g o) -> p g o", o=5),
                    bcast_last(rden[:, :w // 5], 5),
                    mybir.AluOpType.mult)
                seng.dma_start(
                    out_d.ap()[2 * m + lo // 80: 2 * m + hi // 80]
                    .rearrange("t p kc -> p t kc"),
                    probs[:, lo:hi].rearrange("p (t kc) -> p t kc",
                                              t=(hi - lo) // 80))

    nc.compile()
    return nc


def _get_program(teff=T):
    if teff not in _cache:
        _cache[teff] = _build_program(teff)
    return _cache[teff]


# ----------------------------------------------------------------------------
# host-side data prep
# ----------------------------------------------------------------------------

def _expert_blobs(W1, W1_a, W2, W2_a, V, V_a):
    """Per-expert weight blobs (uint16 bf16 bits)."""
    W1c = (W1[None] + W1_a).astype(np.float32)    # [J,Q,H,O+1]
    W2c = (W2[None] + W2_a).astype(np.float32)    # [J,H,H+1]
    Vc = (V[None] + V_a).astype(np.float32)       # [J,Q,O,H+1]

    # waq[j, p, 6s+d, s*64+h] = W1c[j, 2p+s, h, d]: L1 stationary per pair
    wa = np.zeros((J, 3, 12, 128), np.float32)
    for q in range(Q - 1):
        p, s = q // 2, q % 2
        wa[:, p, s * 6:s * 6 + 6, s * 64:(s + 1) * 64] = \
            W1c[:, q].transpose(0, 2, 1)
    # b3 row per expert: col blk*40 + p*10 + s*5 + o = Vc[j, 2p+s, o, 0]
    b3 = np.zeros((J, 80), np.float32)
    for q in range(Q):
        p, s = q // 2, q % 2
        for blk in range(2):
            b3[:, blk * 40 + p * 10 + s * 5: blk * 40 + p * 10 + s * 5 + 5] = \
                Vc[:, q, :, 0]
    wa16 = wa.astype(_bf16).view(np.uint16)
    b316 = b3.astype(_bf16).view(np.uint16)

    wb = np.zeros((J, 128, 168), np.float32)
    w2w = W2c[:, :, 1:].transpose(0, 2, 1)        # [J, i, h2]
    for s in range(2):
        wb[:, s * 64:(s + 1) * 64, s * 64:(s + 1) * 64] = w2w
    for q in range(Q):
        p, s = q // 2, q % 2
        # [J, h2, o] <- Vc[:, q, :, 1:] is [J, o, h2]
        wb[:, s * 64:(s + 1) * 64, 128 + p * 10 + s * 5: 128 + p * 10 + s * 5 + 5] = \
            Vc[:, q, :, 1:].transpose(0, 2, 1)
    wb16 = np.zeros((J, 128, WBW), np.uint16)
    wb16[:, :, :168] = wb.astype(_bf16).view(np.uint16)
    wb16[:, :, 172:300] = wb16[:, :, 0:128]
    b2 = np.concatenate([W2c[:, :, 0], W2c[:, :, 0]], axis=1)  # [J, 128]
    wb16[:, :, 168:170] = b2.astype(np.float32).view(np.uint16).reshape(J, 128, 2)
    # per-macro q6 patch blocks (partner expert's q6 weights, s=1 halves)
    q6w1 = W1c[:, 6].transpose(0, 2, 1).astype(_bf16).view(np.uint16)     # [J,6,64]
    q6v = Vc[:, 6, :, 1:].transpose(0, 2, 1).astype(_bf16).view(np.uint16)  # [J,64,5]
    return wa16, b316, wb16, q6w1, q6v


def _plan(judge_ids):
    """Supertile schedule: list of (judge, sample_idx_array), core/slot map."""
    jid = np.asarray(judge_ids).astype(np.int64).ravel()
    assert jid.shape[0] == B
    order = np.argsort(jid, kind="stable")
    counts = np.bincount(jid, minlength=J)
    tiles = []
    pos = 0
    for j in range(J):
        g = order[pos:pos + counts[j]]
        pos += counts[j]
        for s in range(0, len(g), ST):
            tiles.append((j, g[s:s + ST]))
    assert len(tiles) <= N_CORES * T, f"{len(tiles)} supertiles > capacity"
    return tiles


def _prepare_inputs(x, judge_ids, W1, W1_a, W2, W2_a, V, V_a):
    x = np.ascontiguousarray(np.asarray(x, dtype=np.float32))
    wa16, b316, wb16, q6w1, q6v = _expert_blobs(
        *(np.asarray(a, dtype=np.float32)
          for a in (W1, W1_a, W2, W2_a, V, V_a)))
    tiles = _plan(judge_ids)
    # compile/run for the actual slot count (rounded to whole macros),
    # not the worst-case T
    teff = -(-len(tiles) // N_CORES)
    teff += teff % 2

    judge_mat = np.zeros((N_CORES, teff), np.int64)      # expert per slot
    xg = np.zeros((N_CORES, teff, ST, Q, O), np.float32)  # gathered x
    for i, (j, g) in enumerate(tiles):
        k, t = i % N_CORES, i // N_CORES
        judge_mat[k, t] = j
        xg[k, t, :len(g)] = x[g]

    # xw macro blob [108, XWW]: row group g = partitions 32g..32g+11, one
    # question pair each; group 3 is the shared q6 block (s0 = u0's q6,
    # s1 = u1's q6)
    nm = teff // 2
    xwf = np.zeros((N_CORES, nm, 108, XWW), np.float32)
    for g in range(3):
        for s in range(2):
            r, q = 32 * g + 6 * s, 2 * g + s
            xwf[:, :, r, 0:512] = 1.0
            xwf[:, :, r + 1:r + 6, 0:256] = \
                xg[:, 0::2, :, q, :].transpose(0, 1, 3, 2)
            xwf[:, :, r + 1:r + 6, 256:512] = \
                xg[:, 1::2, :, q, :].transpose(0, 1, 3, 2)
    xwf[:, :, 96, 0:256] = 1.0
    xwf[:, :, 97:102, 0:256] = xg[:, 0::2, :, 6, :].transpose(0, 1, 3, 2)
    xwf[:, :, 102, 0:256] = 1.0
    xwf[:, :, 103:108, 0:256] = xg[:, 1::2, :, 6, :].transpose(0, 1, 3, 2)
    xw16 = xwf.astype(_bf16).view(np.uint16)

    j0, j1 = judge_mat[:, 0::2], judge_mat[:, 1::2]  # experts per macro
    for g in range(3):
        xw16[:, :, 32 * g:32 * g + 12, 512:640] = wa16[j0][:, :, g]
        xw16[:, :, 32 * g:32 * g + 12, 640:768] = wa16[j1][:, :, g]
    # q6 mixed L1 stationary in group 3
    xw16[:, :, 96:102, 512:576] = q6w1[j0]
    xw16[:, :, 102:108, 576:640] = q6w1[j1]
    # both tiles' b3 rows on partition 32
    xw16[:, :, 32, 768:848] = b316[j0]
    xw16[:, :, 32, 848:928] = b316[j1]

    in_maps = []
    for k in range(N_CORES):
        wbs = wb16[judge_mat[k]].copy()
        # partner's q6 V block into the s=1 half of u0's p3 slot
        wbs[0::2, 64:128, 163:168] = q6v[j1[k]]
        # mixed L2 bias [b2(j0); b2(j1)] for the q6 block, f32 bits
        wbs[0::2, 64:128, 170:172] = wbs[1::2, 64:128, 168:170]
        wbs[0::2, 0:64, 170:172] = wbs[0::2, 0:64, 168:170]
        # partner's W2 into the s=1 half of the mixed block
        wbs[0::2, 64:128, 236:300] = wbs[1::2, 64:128, 64:128]
        wbm = np.zeros((teff // 2, 128, 472), np.uint16)
        wbm[:, :, 0:300] = wbs[0::2]
        wbm[:, :, 300:472] = wbs[1::2, :, 0:172]
        in_maps.append({
            "xw": np.ascontiguousarray(xw16[k]).view(_bf16),
            "wb": np.ascontiguousarray(wbm).view(_bf16),
        })
    return in_maps, tiles, teff


def _assemble_output(results, tiles):
    out = np.empty((B, Q, O), np.float32)
    for i, (_, g) in enumerate(tiles):
        k, t = i % N_CORES, i // N_CORES
        blob = results[k]["out"][t].reshape(128, 2, 40)
        rows = blob.transpose(1, 0, 2).reshape(ST, 40)[:len(g), :35]
        out[g] = rows.reshape(len(g), Q, O)
    return out


# ----------------------------------------------------------------------------
# entry point
# ----------------------------------------------------------------------------

def kernel(x, judge_ids, W1, W1_a, W2, W2_a, V, V_a):
    from concourse import bass_utils
    in_maps, tiles, teff = _prepare_inputs(x, judge_ids, W1, W1_a, W2, W2_a, V, V_a)
    nc = _get_program(teff)
    res = bass_utils.run_bass_kernel_spmd(
        nc, in_maps, core_ids=list(range(N_CORES)), trace=False)
    return _assemble_output(res.results, tiles)


# expose for test harness reuse
def run_with_results(x, judge_ids, W1, W1_a, W2, W2_a, V, V_a, trace=False,
                     **kwargs):
    from concourse import bass_utils
    in_maps, tiles, teff = _prepare_inputs(x, judge_ids, W1, W1_a, W2, W2_a, V, V_a)
    nc = _get_program(teff)
    res = bass_utils.run_bass_kernel_spmd(
        nc, in_maps, core_ids=list(range(N_CORES)), trace=trace, **kwargs)
    return _assemble_output(res.results, tiles), res

